# Initial kernel scaffold
#
"""Mixture-of-Experts (top-2 of 8 experts, erf-GELU FFN) on 8 Trainium2
NeuronCores, expert-parallel: core e owns expert e's weights and processes
only the tokens routed to expert e.

Host side (inside kernel()): router softmax + top-2 + renormalized combine
weights, token dispatch (gather per expert) and combine (scatter-add).
Device side (Bass/Tile SPMD): per-core FFN
    y = gelu(xg @ W1[e] + b1[e]) @ W2[e] + b2[e], scaled by combine weight,
with bf16 matmuls and fp32 accumulation.

Layouts shipped per core (P=128 partitions, C = token capacity, Cb=C/128):
  xt  [P, D/128, C]  bf16   xt[p,db,c]  = x_gathered[c, db*128+p]   (x^T)
  w1  [P, D/128, F]  bf16   w1[p,db,f]  = W1[e][db*128+p, f]
  w2  [P, F/128, D]  bf16   w2[p,fb,d]  = W2[e][fb*128+p, d]
  b1  [P, F/128]     f32    b1[p,fb]    = b1[e][fb*128+p]
  b2c [P, D/128]     f32    b2c[p,dt]   = b2[e][dt*128+p]
  wgb [P, C]         f32    combine weight of slot c, broadcast over p
  out [P, D/128, C]  f32    out[p,dt,c] = y[c, dt*128+p]           (y^T)

MM1: hT[F,C] += W1_tile[128(D),128(F)].T @ xT[128(D), C-chunk]  (accum D)
     gelu+b1 fused in the PSUM->SBUF eviction on ScalarE (erf gelu).
MM2: yT[D,C] += w2_tile[128(F),128(D)].T @ hT[128(F), C-chunk]  (accum F)
     tokens stay the moving dim, so PE cost scales with the real C
     instead of padded 128-token tiles. b2 add (per-partition) +
     combine-weight multiply (free-dim row) fused in the eviction.
"""

import numpy as np
import ml_dtypes

P = 128
N_CORES = 8

_cache = {}
_last_in_maps = None


def _build(C, D, F):
    """Build + compile the per-core SPMD Bass program for capacity C."""
    from concourse import bacc
    import concourse.tile as tile
    import concourse.mybir as mybir

    nb_d = D // P          # D-tiles (contraction of MM1, output tiles of MM2)
    nb_f = F // P          # F-tiles (contraction of MM2)

    # C-chunks of <=512 for the MM1 moving operand / PSUM bank.
    # Split evenly: a tiny tail chunk (N << 128) runs LDWEIGHTS-bound on
    # the PE, so two balanced ~C/2 chunks beat (512, C-512).
    n_chunks = (C + 511) // 512
    chunks = []
    c0 = 0
    for i in range(n_chunks):
        cn = (C - c0 + (n_chunks - 1 - i)) // (n_chunks - i)
        chunks.append((c0, cn))
        c0 += cn

    bf16 = mybir.dt.bfloat16
    f32 = mybir.dt.float32
    GELU = mybir.ActivationFunctionType.Gelu

    nc = bacc.Bacc(None, target_bir_lowering=False)
    xt_d = nc.dram_tensor("xt", [P, nb_d, C], bf16, kind="ExternalInput")
    # w1 shipped tile-major [fb, p, db, q] so each per-fb load is one dense
    # per-partition-contiguous transfer (the [P, nb_d, F] layout fragments
    # into 256 B bursts and runs at ~11 GB/s)
    w1_d = nc.dram_tensor("w1", [nb_f, P, nb_d, P], bf16, kind="ExternalInput")
    w2_d = nc.dram_tensor("w2", [P, nb_f, D], bf16, kind="ExternalInput")
    b1_d = nc.dram_tensor("b1", [P, nb_f], f32, kind="ExternalInput")
    b2_d = nc.dram_tensor("b2c", [P, nb_d], f32, kind="ExternalInput")
    wg_d = nc.dram_tensor("wgb", [P, C], f32, kind="ExternalInput")
    out_d = nc.dram_tensor("out", [P, nb_d, C], f32, kind="ExternalOutput")

    with tile.TileContext(nc) as tc:
        with (
            tc.tile_pool(name="const", bufs=1) as const,
            tc.tile_pool(name="w1p", bufs=6) as w1p,
            tc.tile_pool(name="ps1", bufs=3, space="PSUM") as ps1p,
            tc.tile_pool(name="ps2", bufs=4, space="PSUM") as ps2p,
            tc.tile_pool(name="outp", bufs=3) as outp,
        ):
            xt_t = const.tile([P, nb_d, C], bf16)
            b1_t = const.tile([P, nb_f], f32)
            b2_t = const.tile([P, nb_d], f32)
            wg_t = const.tile([P, C], f32)
            w2_t = const.tile([P, nb_f, D], bf16)
            h_t = const.tile([P, nb_f, C], bf16)

            # critical path first: xt gates MM1 — split across the three
            # DMA-capable queues in parallel. Small constants follow on
            # GpSimd. (Starting MM1 before the full xt lands just turns
            # head wait into mid-accumulation PE stalls — measured.)
            splits = [(nc.sync, 0, 3), (nc.gpsimd, 3, 6), (nc.scalar, 6, nb_d)]
            for eng, a, b in splits:
                eng.dma_start(xt_t[:, a:b, :], xt_d[:, a:b, :])
            nc.gpsimd.dma_start(b1_t[:], b1_d[:])
            nc.gpsimd.dma_start(wg_t[:], wg_d[:])
            nc.gpsimd.dma_start(b2_t[:], b2_d[:])

            # ---- MM1: hT[fb] = gelu(sum_db w1[db,fb]^T @ xT[db] + b1[fb])
            # w2 (8 MiB, needed only for MM2) trickles in behind the w1
            # stream: one chunk every 4 fb so it never starves MM1's loads
            for fb in range(nb_f):
                w1_t = w1p.tile([P, nb_d, P], bf16)
                nc.sync.dma_start(w1_t[:], w1_d[fb])
                if fb % 4 == 2:
                    q, qs = fb // 4, nb_f // 8
                    nc.sync.dma_start(
                        w2_t[:, q * qs : (q + 1) * qs, :],
                        w2_d[:, q * qs : (q + 1) * qs, :],
                    )
                for c0, cn in chunks:
                    ps = ps1p.tile([P, 512], f32)
                    for db in range(nb_d):
                        nc.tensor.matmul(
                            ps[:, :cn],
                            lhsT=w1_t[:, db, :],
                            rhs=xt_t[:, db, c0 : c0 + cn],
                            start=(db == 0),
                            stop=(db == nb_d - 1),
                        )
                    nc.scalar.activation(
                        h_t[:, fb, c0 : c0 + cn],
                        ps[:, :cn],
                        GELU,
                        bias=b1_t[:, fb : fb + 1],
                    )

            # ---- MM2: yT[dt] = (sum_fb w2[fb,dt]^T @ hT[fb]) + b2, * wg
            # Tokens are the moving dim: PE cost scales with the real C.
            for dt in range(nb_d):
                o_t = outp.tile([P, C], f32)
                for c0, cn in chunks:
                    ps = ps2p.tile([P, 512], f32)
                    for fb in range(nb_f):
                        nc.tensor.matmul(
                            ps[:, :cn],
                            lhsT=w2_t[:, fb, dt * P : (dt + 1) * P],
                            rhs=h_t[:, fb, c0 : c0 + cn],
                            start=(fb == 0),
                            stop=(fb == nb_f - 1),
                        )
                    sl = slice(c0, c0 + cn)
                    nc.vector.scalar_tensor_tensor(
                        o_t[:, sl],
                        ps[:, :cn],
                        b2_t[:, dt : dt + 1],
                        wg_t[:, sl],
                        op0=mybir.AluOpType.add,
                        op1=mybir.AluOpType.mult,
                    )
                nc.sync.dma_start(out_d[:, dt, :], o_t[:])

    nc.compile()
    return nc


def _route(x, W_router):
    """Top-2 routing, replicating jax softmax/top_k/renorm semantics."""
    T = x.shape[0]
    logits = x @ np.asarray(W_router, np.float32)
    m = logits.max(axis=1, keepdims=True)
    ex = np.exp(logits - m, dtype=np.float32)
    probs = ex / ex.sum(axis=1, keepdims=True, dtype=np.float32)
    r = np.arange(T)
    i1 = probs.argmax(axis=1)
    masked = probs.copy()
    masked[r, i1] = -np.inf
    i2 = masked.argmax(axis=1)
    p1 = probs[r, i1]
    p2 = probs[r, i2]
    s = p1 + p2
    return i1, i2, p1 / s, p2 / s


def kernel(hidden_states, W_router, W1, b1, W2, b2):
    from concourse.bass_utils import run_bass_kernel_spmd

    B, S, D = hidden_states.shape
    E, _, F = W1.shape
    T = B * S
    x = np.ascontiguousarray(np.asarray(hidden_states, np.float32).reshape(T, D))

    i1, i2, w1c, w2c = _route(x, W_router)

    idxs, wgts = [], []
    for e in range(E):
        sel1 = i1 == e
        sel2 = i2 == e
        idx = np.nonzero(sel1 | sel2)[0]
        w = np.where(sel1[idx], w1c[idx], w2c[idx]).astype(np.float32)
        idxs.append(idx)
        wgts.append(w)

    C = max(max(len(ix) for ix in idxs), 1)
    nb_d = D // P
    nb_f = F // P

    key = (C, D, F)
    if key not in _cache:
        _cache[key] = _build(C, D, F)
    nc = _cache[key]

    bf16 = ml_dtypes.bfloat16
    W1b = np.asarray(W1, np.float32).astype(bf16)
    W2b = np.asarray(W2, np.float32).astype(bf16)
    xb = x.astype(bf16)

    in_maps = []
    for e in range(E):
        n = len(idxs[e])
        xg = np.zeros((C, D), bf16)
        xg[:n] = xb[idxs[e]]
        xt = np.ascontiguousarray(xg.T.reshape(nb_d, P, C).transpose(1, 0, 2))
        w1e = np.ascontiguousarray(
            W1b[e].reshape(nb_d, P, nb_f, P).transpose(2, 1, 0, 3)
        )
        w2e = np.ascontiguousarray(W2b[e].reshape(nb_f, P, D).transpose(1, 0, 2))
        b1e = np.ascontiguousarray(np.asarray(b1[e], np.float32).reshape(nb_f, P).T)
        b2e = np.ascontiguousarray(np.asarray(b2[e], np.float32).reshape(nb_d, P).T)
        wfull = np.zeros(C, np.float32)
        wfull[:n] = wgts[e]
        wgb = np.ascontiguousarray(np.broadcast_to(wfull, (P, C)))
        in_maps.append(
            {"xt": xt, "w1": w1e, "w2": w2e, "b1": b1e, "b2c": b2e, "wgb": wgb}
        )

    global _last_in_maps
    _last_in_maps = in_maps

    res = run_bass_kernel_spmd(nc, in_maps, core_ids=list(range(N_CORES)))

    out = np.zeros((T, D), np.float32)
    for e in range(E):
        n = len(idxs[e])
        # device out is y^T tiled [P, nb_d, C]: out[p, dt, c] = y[c, dt*P+p]
        y = (
            np.asarray(res.results[e]["out"])
            .transpose(2, 1, 0)
            .reshape(C, D)[:n]
        )
        out[idxs[e]] += y
    return out.reshape(B, S, D).astype(np.float32)



# revision 7
# speedup vs baseline: 1.2107x; 1.2107x over previous
"""Mixture-of-Experts (top-2 of 8 experts, erf-GELU FFN) on 8 Trainium2
NeuronCores, expert-parallel: core e owns expert e's weights and processes
only the tokens routed to expert e.

Host side (inside kernel()): router softmax + top-2 + renormalized combine
weights, token dispatch (gather per expert) and combine (scatter-add).
Device side (Bass/Tile SPMD): per-core FFN
    y = gelu(xg @ W1[e] + b1[e]) @ W2[e] + b2[e], scaled by combine weight,
with bf16 matmuls and fp32 accumulation.

v2 schedule notes (from ntff trace of v1):
  - exec time = head + PE stream + tail; the stream (282k moving columns)
    runs at the PE roofline with zero stalls, so v2 attacks head/tail:
  - head: v1 serialized xt+w1+w2 on one DMA queue (first matmul at 15.2us).
    v2 dedicates sync queue to w1 (fb0 first, alone), ships xt as two
    contiguous chunk tiles on scalar/gpsimd, w2 on gpsimd.
  - cold-start: HAM keeps PE at 1.2 GHz until ~3.4us of sustained busy.
    v2 issues 9 dummy 512-col matmuls on a zeroed tile at kernel start so
    the real stream begins warm.
  - ACT tables: a dummy 1-col gelu pulls the ~3us ACT_TABLE_LOAD into the
    DMA head instead of the first PSUM eviction.
  - tail: the NEFF epilogue spends ~155ns per DMA instruction; v2 cuts
    DMA instruction count 54 -> ~21 (w1 in 4-fb groups, w2 in 4 chunks,
    merged consts, paired bf16 outputs).

Layouts shipped per core (P=128 partitions, C = token capacity padded to
2*CH, CH = chunk length):
  xt0/xt1 [P, D/128, CH] bf16   xt{c}[p,db,j] = x_gathered[c*CH+j, db*128+p]
  w1a [P, 1, D/128, P]  bf16    fb=0 tile      (w1 tile-major: [p, db, q] =
  w1b [P, 3, D/128, P]  bf16    fb=1..3         W1[e][db*128+p, fb*128+q])
  w1g [7, P, 4, D/128, P] bf16  fb=4..31 in groups of 4
  w2  [P, F/128, D]     bf16    w2[p,fb,d] = W2[e][fb*128+p, d]
  b1  [P, F/128]        f32     b1[p,fb]   = b1[e][fb*128+p]
  cw  [P, C+D/128]      f32     [:, :C] combine weight (bcast over p),
                                [:, C+dt] = b2[e][dt*128+p]
  out [ND2, P, 2, C]    bf16    out[i,p,j,c] = y[c, (2i+j)*128+p]
"""

import numpy as np
import ml_dtypes

P = 128
N_CORES = 8

_cache = {}
_last_in_maps = None


def _build(C, D, F):
    """Build + compile the per-core SPMD Bass program for padded capacity C."""
    from concourse import bacc
    import concourse.tile as tile
    import concourse.mybir as mybir

    nb_d = D // P          # D-tiles (contraction of MM1, output tiles of MM2)
    nb_f = F // P          # F-tiles (contraction of MM2)

    n_chunks = (C + 511) // 512
    CH = C // n_chunks
    assert CH * n_chunks == C and CH % 4 == 0
    chunks = [(i * CH, CH) for i in range(n_chunks)]

    bf16 = mybir.dt.bfloat16
    f32 = mybir.dt.float32
    GELU = mybir.ActivationFunctionType.Gelu

    # w1 group structure: fb0 alone (unblocks the first matmul ASAP),
    # fb1-3, then groups of 4.
    groups = [(0, 1), (1, 3)] + [(s, 4) for s in range(4, nb_f, 4)]

    nc = bacc.Bacc(None, target_bir_lowering=False)
    xt_d = [
        nc.dram_tensor(f"xt{i}", [P, nb_d, CH], bf16, kind="ExternalInput")
        for i in range(n_chunks)
    ]
    w1_d = [
        nc.dram_tensor(f"w1_{i}", [P, n, nb_d, P], bf16, kind="ExternalInput")
        for i, (_, n) in enumerate(groups)
    ]
    w2_d = nc.dram_tensor("w2", [P, nb_f, D], bf16, kind="ExternalInput")
    b1_d = nc.dram_tensor("b1", [P, nb_f], f32, kind="ExternalInput")
    cw_d = nc.dram_tensor("cw", [P, C + nb_d], f32, kind="ExternalInput")
    nd2 = (nb_d + 1) // 2
    out_d = nc.dram_tensor("out", [nd2, P, 2, C], bf16, kind="ExternalOutput")

    with tile.TileContext(nc) as tc:
        with (
            tc.tile_pool(name="const", bufs=1) as const,
            tc.tile_pool(name="w1p", bufs=3) as w1p,
            tc.tile_pool(name="ps1", bufs=3, space="PSUM") as ps1p,
            tc.tile_pool(name="ps2", bufs=4, space="PSUM") as ps2p,
            tc.tile_pool(name="pwm", bufs=1, space="PSUM") as pwmp,
            tc.tile_pool(name="outp", bufs=3) as outp,
        ):
            zt = const.tile([P, 512], bf16)
            xt_t = [
                const.tile([P, nb_d, CH], bf16, name=f"xt{i}_t")
                for i in range(n_chunks)
            ]
            b1_t = const.tile([P, nb_f], f32)
            cw_t = const.tile([P, C + nb_d], f32)
            w2_t = const.tile([P, nb_f, D], bf16)
            h_t = const.tile([P, nb_f, C], bf16)
            w1ab = [
                const.tile([P, n, nb_d, P], bf16, name=f"w1ab{n}_t")
                for _, n in groups[:2]
            ]
            dum = const.tile([P, 1], f32)

            # ---- PE warm-up: HAM un-throttles (1.2 -> 2.4 GHz) only after
            # ~3.4us of sustained matmul activity. Run ~3.8us of dummy
            # 512-col matmuls on a zeroed tile, concurrent with the input
            # DMA head, so the real stream starts warm.
            nc.gpsimd.memset(zt[:], 0.0)
            pw = pwmp.tile([P, 512], f32)
            for _ in range(9):
                nc.tensor.matmul(
                    pw[:], lhsT=zt[:, :P], rhs=zt[:], start=True, stop=True
                )

            # ---- input DMA: per-queue issue order is the schedule.
            # sync (HWDGE): the w1 stream, fb0 first and alone.
            # scalar (HWDGE): xt chunk0 (gates first matmul), then the
            #   gelu-table preload (dummy activation).
            # gpsimd (SWDGE): xt chunk1, b1, cw, then w2 in 4 chunks.
            for i in range(2):
                nc.sync.dma_start(w1ab[i][:], w1_d[i][:])
            w1g_t = []
            for i in range(2, len(groups)):
                g = w1p.tile([P, 4, nb_d, P], bf16, name="w1g_t")
                nc.sync.dma_start(g[:], w1_d[i][:])
                w1g_t.append(g)

            nc.scalar.dma_start(xt_t[0][:], xt_d[0][:])
            nc.scalar.activation(dum[:], zt[:, :1], GELU)

            for i in range(1, n_chunks):
                nc.gpsimd.dma_start(xt_t[i][:], xt_d[i][:])
            nc.gpsimd.dma_start(b1_t[:], b1_d[:])
            nc.gpsimd.dma_start(cw_t[:], cw_d[:])
            qf = nb_f // 4
            for q in range(4):
                nc.gpsimd.dma_start(
                    w2_t[:, q * qf : (q + 1) * qf, :],
                    w2_d[:, q * qf : (q + 1) * qf, :],
                )

            # ---- MM1: hT[fb] = gelu(sum_db w1[db,fb]^T @ xT[db] + b1[fb])
            def w1_tile(fb):
                if fb == 0:
                    return w1ab[0][:, 0]
                if fb < 4:
                    return w1ab[1][:, fb - 1]
                return w1g_t[(fb - 4) // 4][:, fb % 4]

            for fb in range(nb_f):
                wt = w1_tile(fb)
                for ci, (c0, cn) in enumerate(chunks):
                    ps = ps1p.tile([P, 512], f32)
                    for db in range(nb_d):
                        nc.tensor.matmul(
                            ps[:, :cn],
                            lhsT=wt[:, db, :],
                            rhs=xt_t[ci][:, db, :],
                            start=(db == 0),
                            stop=(db == nb_d - 1),
                        )
                    nc.scalar.activation(
                        h_t[:, fb, c0 : c0 + cn],
                        ps[:, :cn],
                        GELU,
                        bias=b1_t[:, fb : fb + 1],
                    )

            # ---- MM2: yT[dt] = (sum_fb w2[fb,dt]^T @ hT[fb]) + b2, * wg
            # bf16 output, DMA'd per dt-pair to cut instruction count.
            for dt in range(nb_d):
                if dt % 2 == 0:
                    o_t = outp.tile([P, 2, C], bf16)
                for c0, cn in chunks:
                    ps = ps2p.tile([P, 512], f32)
                    for fb in range(nb_f):
                        nc.tensor.matmul(
                            ps[:, :cn],
                            lhsT=w2_t[:, fb, dt * P : (dt + 1) * P],
                            rhs=h_t[:, fb, c0 : c0 + cn],
                            start=(fb == 0),
                            stop=(fb == nb_f - 1),
                        )
                    nc.vector.scalar_tensor_tensor(
                        o_t[:, dt % 2, c0 : c0 + cn],
                        ps[:, :cn],
                        cw_t[:, C + dt : C + dt + 1],
                        cw_t[:, c0 : c0 + cn],
                        op0=mybir.AluOpType.add,
                        op1=mybir.AluOpType.mult,
                    )
                if dt % 2 == 1:
                    nc.sync.dma_start(out_d[dt // 2], o_t[:])

    nc.compile()
    return nc


def _route(x, W_router):
    """Top-2 routing, replicating jax softmax/top_k/renorm semantics."""
    T = x.shape[0]
    logits = x @ np.asarray(W_router, np.float32)
    m = logits.max(axis=1, keepdims=True)
    ex = np.exp(logits - m, dtype=np.float32)
    probs = ex / ex.sum(axis=1, keepdims=True, dtype=np.float32)
    r = np.arange(T)
    i1 = probs.argmax(axis=1)
    masked = probs.copy()
    masked[r, i1] = -np.inf
    i2 = masked.argmax(axis=1)
    p1 = probs[r, i1]
    p2 = probs[r, i2]
    s = p1 + p2
    return i1, i2, p1 / s, p2 / s


def kernel(hidden_states, W_router, W1, b1, W2, b2):
    from concourse.bass_utils import run_bass_kernel_spmd

    B, S, D = hidden_states.shape
    E, _, F = W1.shape
    T = B * S
    x = np.ascontiguousarray(np.asarray(hidden_states, np.float32).reshape(T, D))

    i1, i2, w1c, w2c = _route(x, W_router)

    idxs, wgts = [], []
    for e in range(E):
        sel1 = i1 == e
        sel2 = i2 == e
        idx = np.nonzero(sel1 | sel2)[0]
        w = np.where(sel1[idx], w1c[idx], w2c[idx]).astype(np.float32)
        idxs.append(idx)
        wgts.append(w)

    Craw = max(max(len(ix) for ix in idxs), 1)
    nb_d = D // P
    nb_f = F // P
    n_chunks = (Craw + 511) // 512
    CH = -(-Craw // (n_chunks * 4)) * 4     # chunk len, multiple of 4
    C = CH * n_chunks                        # padded capacity

    key = (C, D, F)
    if key not in _cache:
        _cache[key] = _build(C, D, F)
    nc = _cache[key]

    bf16 = ml_dtypes.bfloat16
    W1b = np.asarray(W1, np.float32).astype(bf16)
    W2b = np.asarray(W2, np.float32).astype(bf16)
    xb = x.astype(bf16)

    groups = [(0, 1), (1, 3)] + [(s, 4) for s in range(4, nb_f, 4)]
    nd2 = (nb_d + 1) // 2

    in_maps = []
    for e in range(E):
        n = len(idxs[e])
        xg = np.zeros((C, D), bf16)
        xg[:n] = xb[idxs[e]]
        m = {}
        for i in range(n_chunks):
            xc = xg[i * CH : (i + 1) * CH]  # [CH, D]
            m[f"xt{i}"] = np.ascontiguousarray(
                xc.T.reshape(nb_d, P, CH).transpose(1, 0, 2)
            )
        # w1 tile-major, grouped: [p, fb-in-group, db, q]
        w1all = W1b[e].reshape(nb_d, P, nb_f, P).transpose(2, 1, 0, 3)
        for i, (s, ng) in enumerate(groups):
            m[f"w1_{i}"] = np.ascontiguousarray(
                w1all[s : s + ng].transpose(1, 0, 2, 3)
            )
        m["w2"] = np.ascontiguousarray(W2b[e].reshape(nb_f, P, D).transpose(1, 0, 2))
        m["b1"] = np.ascontiguousarray(np.asarray(b1[e], np.float32).reshape(nb_f, P).T)
        cw = np.zeros((P, C + nb_d), np.float32)
        cw[:, :n] = wgts[e]
        cw[:, C:] = np.asarray(b2[e], np.float32).reshape(nb_d, P).T
        m["cw"] = cw
        in_maps.append(m)

    global _last_in_maps
    _last_in_maps = in_maps

    res = run_bass_kernel_spmd(nc, in_maps, core_ids=list(range(N_CORES)))

    out = np.zeros((T, D), np.float32)
    for e in range(E):
        n = len(idxs[e])
        # device out[i, p, j, c] = y[c, (2i+j)*128+p]
        y = (
            np.asarray(res.results[e]["out"])
            .astype(np.float32)
            .transpose(3, 0, 2, 1)
            .reshape(C, nd2 * 2 * P)[:n, :D]
        )
        out[idxs[e]] += y
    return out.reshape(B, S, D).astype(np.float32)


# revision 9
# speedup vs baseline: 1.2228x; 1.0100x over previous
"""Mixture-of-Experts (top-2 of 8 experts, erf-GELU FFN) on 8 Trainium2
NeuronCores, expert-parallel: core e owns expert e's weights and processes
only the tokens routed to expert e.

Host side (inside kernel()): router softmax + top-2 + renormalized combine
weights, token dispatch (gather per expert) and combine (scatter-add).
Device side (Bass/Tile SPMD): per-core FFN
    y = gelu(xg @ W1[e] + b1[e]) @ W2[e] + b2[e], scaled by combine weight,
with bf16 matmuls and fp32 accumulation.

v2 schedule notes (from ntff trace of v1):
  - exec time = head + PE stream + tail; the stream (282k moving columns)
    runs at the PE roofline with zero stalls, so v2 attacks head/tail:
  - head: v1 serialized xt+w1+w2 on one DMA queue (first matmul at 15.2us).
    v2 dedicates sync queue to w1 (fb0 first, alone), ships xt as two
    contiguous chunk tiles on scalar/gpsimd, w2 on gpsimd.
  - cold-start: HAM keeps PE at 1.2 GHz until ~3.4us of sustained busy.
    v2 issues 9 dummy 512-col matmuls on a zeroed tile at kernel start so
    the real stream begins warm.
  - ACT tables: a dummy 1-col gelu pulls the ~3us ACT_TABLE_LOAD into the
    DMA head instead of the first PSUM eviction.
  - tail: the NEFF epilogue spends ~155ns per DMA instruction; v2 cuts
    DMA instruction count 54 -> ~21 (w1 in 4-fb groups, w2 in 4 chunks,
    merged consts, paired bf16 outputs).

Layouts shipped per core (P=128 partitions, C = token capacity padded to
2*CH, CH = chunk length):
  xt0/xt1 [P, D/128, CH] bf16   xt{c}[p,db,j] = x_gathered[c*CH+j, db*128+p]
  w1a [P, 1, D/128, P]  bf16    fb=0 tile      (w1 tile-major: [p, db, q] =
  w1b [P, 3, D/128, P]  bf16    fb=1..3         W1[e][db*128+p, fb*128+q])
  w1g [7, P, 4, D/128, P] bf16  fb=4..31 in groups of 4
  w2  [P, F/128, D]     bf16    w2[p,fb,d] = W2[e][fb*128+p, d]
  b1  [P, F/128]        f32     b1[p,fb]   = b1[e][fb*128+p]
  cw  [P, C+D/128]      f32     [:, :C] combine weight (bcast over p),
                                [:, C+dt] = b2[e][dt*128+p]
  out [ND2, P, 2, C]    bf16    out[i,p,j,c] = y[c, (2i+j)*128+p]
"""

import numpy as np
import ml_dtypes

P = 128
N_CORES = 8

_cache = {}
_last_in_maps = None


def _build(C, D, F):
    """Build + compile the per-core SPMD Bass program for padded capacity C."""
    from concourse import bacc
    import concourse.tile as tile
    import concourse.mybir as mybir

    nb_d = D // P          # D-tiles (contraction of MM1, output tiles of MM2)
    nb_f = F // P          # F-tiles (contraction of MM2)

    n_chunks = (C + 511) // 512
    CH = C // n_chunks
    assert CH * n_chunks == C and CH % 4 == 0
    chunks = [(i * CH, CH) for i in range(n_chunks)]

    bf16 = mybir.dt.bfloat16
    f32 = mybir.dt.float32
    GELU = mybir.ActivationFunctionType.Gelu

    # w1 group structure: fb0 alone (unblocks the first matmul ASAP),
    # fb1-3, then groups of 4.
    groups = [(0, 1), (1, 3)] + [(s, 4) for s in range(4, nb_f, 4)]

    nc = bacc.Bacc(None, target_bir_lowering=False)
    xt_d = [
        nc.dram_tensor(f"xt{i}", [P, nb_d, CH], bf16, kind="ExternalInput")
        for i in range(n_chunks)
    ]
    w1_d = [
        nc.dram_tensor(f"w1_{i}", [P, n, nb_d, P], bf16, kind="ExternalInput")
        for i, (_, n) in enumerate(groups)
    ]
    w2_d = nc.dram_tensor("w2", [P, nb_f, D], bf16, kind="ExternalInput")
    b1_d = nc.dram_tensor("b1", [P, nb_f], f32, kind="ExternalInput")
    cw_d = nc.dram_tensor("cw", [P, C + nb_d], f32, kind="ExternalInput")
    nd2 = (nb_d + 1) // 2
    out_d = nc.dram_tensor("out", [nd2, P, 2, C], bf16, kind="ExternalOutput")

    with tile.TileContext(nc) as tc:
        with (
            tc.tile_pool(name="const", bufs=1) as const,
            tc.tile_pool(name="w1p", bufs=3) as w1p,
            tc.tile_pool(name="ps1", bufs=3, space="PSUM") as ps1p,
            tc.tile_pool(name="ps2", bufs=4, space="PSUM") as ps2p,
            tc.tile_pool(name="pwm", bufs=1, space="PSUM") as pwmp,
            tc.tile_pool(name="outp", bufs=3) as outp,
        ):
            zt = const.tile([P, 512], bf16)
            xt_t = [
                const.tile([P, nb_d, CH], bf16, name=f"xt{i}_t")
                for i in range(n_chunks)
            ]
            b1_t = const.tile([P, nb_f], f32)
            cw_t = const.tile([P, C + nb_d], f32)
            w2_t = const.tile([P, nb_f, D], bf16)
            h_t = const.tile([P, nb_f, C], bf16)
            w1ab = [
                const.tile([P, n, nb_d, P], bf16, name=f"w1ab{n}_t")
                for _, n in groups[:2]
            ]
            dum = const.tile([P, 1], f32)

            # ---- PE warm-up: HAM un-throttles (1.2 -> 2.4 GHz) only after
            # ~3.4us of sustained matmul activity. Run ~3.4us of dummy
            # 512-col matmuls on a zeroed tile, concurrent with the input
            # DMA head, so the real stream starts warm. memset on Vector
            # (otherwise idle, earliest past the preamble barrier).
            nc.vector.memset(zt[:], 0.0)
            pw = pwmp.tile([P, 512], f32)
            for _ in range(8):
                nc.tensor.matmul(
                    pw[:], lhsT=zt[:, :P], rhs=zt[:], start=True, stop=True
                )

            # ---- input DMA: per-queue issue order is the schedule.
            # The HBM ceiling (~330 GB/s) is shared per-packet across busy
            # queues, so w1 (143 GB/s sustained demand during MM1) rides
            # TWO queues (groups alternate sync/scalar); w2's first half
            # streams on gpsimd, its second half is appended to sync's
            # FIFO after the w1 groups, where the w1 pool's buffer gating
            # schedules it into late-MM1 queue idle.
            # sync:   w1a, g0, g2, g4, g6, w2[16:32], out x4 (later)
            # scalar: w1b, gelu-table preload, g1, g3, g5
            # gpsimd: xt0, xt1, b1, cw, w2[0:16]
            w1g_t = []

            def w1_dma(i, eng):
                g = w1p.tile([P, 4, nb_d, P], bf16, name="w1g_t")
                eng.dma_start(g[:], w1_d[i][:])
                w1g_t.append(g)

            nc.sync.dma_start(w1ab[0][:], w1_d[0][:])
            nc.scalar.dma_start(w1ab[1][:], w1_d[1][:])
            nc.scalar.activation(dum[:], zt[:, :1], GELU)
            for i in range(2, len(groups)):
                w1_dma(i, nc.sync if i % 2 == 0 else nc.scalar)

            for i in range(n_chunks):
                nc.gpsimd.dma_start(xt_t[i][:], xt_d[i][:])
            nc.gpsimd.dma_start(b1_t[:], b1_d[:])
            nc.gpsimd.dma_start(cw_t[:], cw_d[:])
            hf = nb_f // 2
            for q in range(4):
                nc.gpsimd.dma_start(
                    w2_t[:, q * 4 : (q + 1) * 4, :],
                    w2_d[:, q * 4 : (q + 1) * 4, :],
                )
            nc.sync.dma_start(w2_t[:, hf:, :], w2_d[:, hf:, :])

            # ---- MM1: hT[fb] = gelu(sum_db w1[db,fb]^T @ xT[db] + b1[fb])
            def w1_tile(fb):
                if fb == 0:
                    return w1ab[0][:, 0]
                if fb < 4:
                    return w1ab[1][:, fb - 1]
                return w1g_t[(fb - 4) // 4][:, fb % 4]

            for fb in range(nb_f):
                wt = w1_tile(fb)
                for ci, (c0, cn) in enumerate(chunks):
                    ps = ps1p.tile([P, 512], f32)
                    for db in range(nb_d):
                        nc.tensor.matmul(
                            ps[:, :cn],
                            lhsT=wt[:, db, :],
                            rhs=xt_t[ci][:, db, :],
                            start=(db == 0),
                            stop=(db == nb_d - 1),
                        )
                    nc.scalar.activation(
                        h_t[:, fb, c0 : c0 + cn],
                        ps[:, :cn],
                        GELU,
                        bias=b1_t[:, fb : fb + 1],
                    )

            # ---- MM2: yT[dt] = (sum_fb w2[fb,dt]^T @ hT[fb]) + b2, * wg
            # bf16 output, DMA'd per dt-pair to cut instruction count.
            for dt in range(nb_d):
                if dt % 2 == 0:
                    o_t = outp.tile([P, 2, C], bf16)
                for c0, cn in chunks:
                    ps = ps2p.tile([P, 512], f32)
                    for fb in range(nb_f):
                        nc.tensor.matmul(
                            ps[:, :cn],
                            lhsT=w2_t[:, fb, dt * P : (dt + 1) * P],
                            rhs=h_t[:, fb, c0 : c0 + cn],
                            start=(fb == 0),
                            stop=(fb == nb_f - 1),
                        )
                    nc.vector.scalar_tensor_tensor(
                        o_t[:, dt % 2, c0 : c0 + cn],
                        ps[:, :cn],
                        cw_t[:, C + dt : C + dt + 1],
                        cw_t[:, c0 : c0 + cn],
                        op0=mybir.AluOpType.add,
                        op1=mybir.AluOpType.mult,
                    )
                if dt % 2 == 1:
                    nc.sync.dma_start(out_d[dt // 2], o_t[:])

    nc.compile()
    return nc


def _route(x, W_router):
    """Top-2 routing, replicating jax softmax/top_k/renorm semantics."""
    T = x.shape[0]
    logits = x @ np.asarray(W_router, np.float32)
    m = logits.max(axis=1, keepdims=True)
    ex = np.exp(logits - m, dtype=np.float32)
    probs = ex / ex.sum(axis=1, keepdims=True, dtype=np.float32)
    r = np.arange(T)
    i1 = probs.argmax(axis=1)
    masked = probs.copy()
    masked[r, i1] = -np.inf
    i2 = masked.argmax(axis=1)
    p1 = probs[r, i1]
    p2 = probs[r, i2]
    s = p1 + p2
    return i1, i2, p1 / s, p2 / s


def kernel(hidden_states, W_router, W1, b1, W2, b2):
    from concourse.bass_utils import run_bass_kernel_spmd

    B, S, D = hidden_states.shape
    E, _, F = W1.shape
    T = B * S
    x = np.ascontiguousarray(np.asarray(hidden_states, np.float32).reshape(T, D))

    i1, i2, w1c, w2c = _route(x, W_router)

    idxs, wgts = [], []
    for e in range(E):
        sel1 = i1 == e
        sel2 = i2 == e
        idx = np.nonzero(sel1 | sel2)[0]
        w = np.where(sel1[idx], w1c[idx], w2c[idx]).astype(np.float32)
        idxs.append(idx)
        wgts.append(w)

    Craw = max(max(len(ix) for ix in idxs), 1)
    nb_d = D // P
    nb_f = F // P
    n_chunks = (Craw + 511) // 512
    CH = -(-Craw // (n_chunks * 4)) * 4     # chunk len, multiple of 4
    C = CH * n_chunks                        # padded capacity

    key = (C, D, F)
    if key not in _cache:
        _cache[key] = _build(C, D, F)
    nc = _cache[key]

    bf16 = ml_dtypes.bfloat16
    W1b = np.asarray(W1, np.float32).astype(bf16)
    W2b = np.asarray(W2, np.float32).astype(bf16)
    xb = x.astype(bf16)

    groups = [(0, 1), (1, 3)] + [(s, 4) for s in range(4, nb_f, 4)]
    nd2 = (nb_d + 1) // 2

    in_maps = []
    for e in range(E):
        n = len(idxs[e])
        xg = np.zeros((C, D), bf16)
        xg[:n] = xb[idxs[e]]
        m = {}
        for i in range(n_chunks):
            xc = xg[i * CH : (i + 1) * CH]  # [CH, D]
            m[f"xt{i}"] = np.ascontiguousarray(
                xc.T.reshape(nb_d, P, CH).transpose(1, 0, 2)
            )
        # w1 tile-major, grouped: [p, fb-in-group, db, q]
        w1all = W1b[e].reshape(nb_d, P, nb_f, P).transpose(2, 1, 0, 3)
        for i, (s, ng) in enumerate(groups):
            m[f"w1_{i}"] = np.ascontiguousarray(
                w1all[s : s + ng].transpose(1, 0, 2, 3)
            )
        m["w2"] = np.ascontiguousarray(W2b[e].reshape(nb_f, P, D).transpose(1, 0, 2))
        m["b1"] = np.ascontiguousarray(np.asarray(b1[e], np.float32).reshape(nb_f, P).T)
        cw = np.zeros((P, C + nb_d), np.float32)
        cw[:, :n] = wgts[e]
        cw[:, C:] = np.asarray(b2[e], np.float32).reshape(nb_d, P).T
        m["cw"] = cw
        in_maps.append(m)

    global _last_in_maps
    _last_in_maps = in_maps

    res = run_bass_kernel_spmd(nc, in_maps, core_ids=list(range(N_CORES)))

    out = np.zeros((T, D), np.float32)
    for e in range(E):
        n = len(idxs[e])
        # device out[i, p, j, c] = y[c, (2i+j)*128+p]
        y = (
            np.asarray(res.results[e]["out"])
            .astype(np.float32)
            .transpose(3, 0, 2, 1)
            .reshape(C, nd2 * 2 * P)[:n, :D]
        )
        out[idxs[e]] += y
    return out.reshape(B, S, D).astype(np.float32)
